# revision 7
# baseline (speedup 1.0000x reference)
"""Trainium2 Bass kernel for BatchRankingLoss.

Reference computation (B=131072, d=256 decoys, K=512 complexes, G=K-1=511 groups):
    o, t -> reshape to [G, d]
    dt = t_i - t_j ; y = sign-ish(dt); w = |dt| > 0.1
    dL = w * max(0, 1 + y*(o_i - o_j));  loss = sum(dL) / (G*d*(d-1))

Key identity used on device: dL is symmetric in (i,j) for |dt|>0.1 pairs, so
    sum(dL) = 2 * sum_{(i,j): dt_ij > 0.1} relu(1 + o_i - o_j)

Device computation per core (64 groups/core, group axis sharded over 8 cores):
  partition p in [0,128) = (g_local = p//2, half = p%2); per partition the free
  axis enumerates (i_local in [0,128), j in [0,256)) = 32768 elements, walked in
  32 chunks of [128, 1024] (each chunk = two N=512 matmul slices).

  - PE:  u   = (t_i - 0.1) - t_j  via K=66 block-diagonal matmul -> PSUM
         do1 = (1 + o_i) - o_j    via K=66 block-diagonal matmul -> PSUM
         (weights [66,128] per slice: rows 0-1 = t'/o' values for the slice's
          two i_local values, rows 2-65 = group-indicator; moving operand fixed
          [66,512]: rows 0-1 = i-slot indicators, rows 2-65 = -t_j / -o_j rows)
  - ACT: s = sigmoid(-1e9 * u)  (exact 0/1 mask away from a ~1e-8 band),
         accum_out -> sig_acc column
  - DVE: m = max(16*s, do1) via scalar_tensor_tensor, accum_out -> m_acc column
  Host:  sum(m) - 16*sum(s) == sum relu(do1 - 16*s) == masked hinge sum,
         since relu(x - S) = max(x, S) - S and s is exactly 0/1.

Host gathers the 8 cores' [128,32] accumulators: loss = 2*(sum m - 16*sum s)/N.
"""

import numpy as np
from contextlib import ExitStack

import concourse.bacc as bacc
import concourse.mybir as mybir
import concourse.tile as tile
from concourse.bass_utils import run_bass_kernel_spmd

N_CORES = 8
D = 256                 # decoys per complex
G_REAL = 511            # torch loop skips the final group
G_PAD = 512             # pad with a zero group so every core gets 64
GPC = G_PAD // N_CORES  # 64 groups per core
P = 128                 # partitions = GPC * 2 halves
IPB = 128               # i_local values per partition (= D/2)
N_SLICES = 64           # 512-wide matmul slices per sweep (2 i_local each)
N_CHUNKS = 32           # [128, 1024] chunks (2 slices each)
KDIM = GPC + 2          # matmul contraction: 64 group rows + 2 i-slot rows
BIG = 1.0e9
CAP = 16.0              # mask magnitude; > max |1 + o_i - o_j|
N_PAIRS = G_REAL * D * (D - 1)

_CACHED = {}


def _build_program(repeat=1):
    """Build the SPMD program. repeat>1 re-runs the compute loop in-NEFF
    (identical work, same outputs) for wall-clock delta timing."""
    nc = bacc.Bacc("TRN2", target_bir_lowering=False, debug=False,
                   num_devices=N_CORES)
    f32 = mybir.dt.float32

    # Per-core external inputs (host-prepped layouts).
    tp2 = nc.dram_tensor("t_part2", [2, N_SLICES * P], f32, kind="ExternalInput")
    op2 = nc.dram_tensor("o_part2", [2, N_SLICES * P], f32, kind="ExternalInput")
    gind = nc.dram_tensor("g_ind", [GPC, P], f32, kind="ExternalInput")
    rhs_t_d = nc.dram_tensor("rhs_t", [KDIM, 512], f32, kind="ExternalInput")
    rhs_o_d = nc.dram_tensor("rhs_o", [KDIM, 512], f32, kind="ExternalInput")

    m_acc_d = nc.dram_tensor("m_acc", [P, N_CHUNKS], f32, kind="ExternalOutput")
    s_acc_d = nc.dram_tensor("s_acc", [P, N_CHUNKS], f32, kind="ExternalOutput")

    with ExitStack() as ctx:
        tc = ctx.enter_context(tile.TileContext(nc, num_cores=N_CORES))
        consts = ctx.enter_context(tc.tile_pool(name="consts", bufs=1))
        psum_u = ctx.enter_context(tc.tile_pool(name="pu", bufs=2, space="PSUM"))
        psum_do = ctx.enter_context(tc.tile_pool(name="pdo", bufs=2, space="PSUM"))
        s_pool = ctx.enter_context(tc.tile_pool(name="sp", bufs=3))
        m_pool = ctx.enter_context(tc.tile_pool(name="mp", bufs=2))

        w_t = consts.tile([KDIM, N_SLICES * P], f32)
        w_o = consts.tile([KDIM, N_SLICES * P], f32)
        rhs_t = consts.tile([KDIM, 512], f32)
        rhs_o = consts.tile([KDIM, 512], f32)
        g_small = consts.tile([GPC, P], f32)
        m_acc = consts.tile([P, N_CHUNKS], f32)
        s_acc = consts.tile([P, N_CHUNKS], f32)

        nc.sync.dma_start(g_small[:], gind[:])
        nc.sync.dma_start(rhs_t[:], rhs_t_d[:])
        nc.sync.dma_start(rhs_o[:], rhs_o_d[:])
        nc.sync.dma_start(w_t[GPC:KDIM, :], tp2[:])
        nc.sync.dma_start(w_o[GPC:KDIM, :], op2[:])

        # Replicate the fixed group-indicator block across all 64 slice
        # positions of each weight buffer (stride-0 read AP), split in half so
        # early matmuls only wait on the first piece.
        HREP = N_SLICES // 2
        for hh in range(2):
            src = g_small[:, None, :].broadcast_to((GPC, HREP, P))
            dst_t = w_t[0:GPC, hh * HREP * P:(hh + 1) * HREP * P]
            dst_o = w_o[0:GPC, hh * HREP * P:(hh + 1) * HREP * P]
            nc.vector.tensor_copy(dst_t.rearrange("g (r p) -> g r p", p=P), src)
            nc.scalar.copy(dst_o.rearrange("g (r p) -> g r p", p=P), src)

        for c in range(N_CHUNKS * repeat):
            c = c % N_CHUNKS
            u_t = psum_u.tile([P, 1024], f32, tag="u")
            do_t = psum_do.tile([P, 1024], f32, tag="do")
            for h in range(2):
                s = 2 * c + h
                nc.tensor.matmul(
                    u_t[:, h * 512:(h + 1) * 512],
                    lhsT=w_t[:, s * P:(s + 1) * P],
                    rhs=rhs_t[:],
                    start=True, stop=True,
                )
            for h in range(2):
                s = 2 * c + h
                nc.tensor.matmul(
                    do_t[:, h * 512:(h + 1) * 512],
                    lhsT=w_o[:, s * P:(s + 1) * P],
                    rhs=rhs_o[:],
                    start=True, stop=True,
                )
            s_t = s_pool.tile([P, 1024], f32, tag="s")
            nc.scalar.activation(
                s_t[:], u_t[:], mybir.ActivationFunctionType.Sigmoid,
                scale=-BIG, accum_out=s_acc[:, c:c + 1],
            )
            m_t = m_pool.tile([P, 1024], f32, tag="m")
            nc.vector.scalar_tensor_tensor(
                out=m_t[:], in0=s_t[:], scalar=CAP, in1=do_t[:],
                op0=mybir.AluOpType.mult, op1=mybir.AluOpType.max,
                accum_out=m_acc[:, c:c + 1],
            )

        nc.sync.dma_start(m_acc_d[:], m_acc[:])
        nc.sync.dma_start(s_acc_d[:], s_acc[:])

    nc.compile()
    return nc


def _prep_core_inputs(t_groups, o_groups):
    """Build per-core input arrays from [GPC, D] group slabs (float32)."""
    tp = t_groups - np.float32(0.1)   # fold the -0.1 threshold into t_i
    op = o_groups + np.float32(1.0)   # fold the +1 hinge margin into o_i

    # [128, 128]: row p=(g_local*2+half) holds i_local values of that half.
    t_part = tp.reshape(GPC * 2, IPB)
    o_part = op.reshape(GPC * 2, IPB)
    # [2, N_SLICES*P]: row k, col s*P+p = value at (p, i_local=2s+k).
    tp2 = np.ascontiguousarray(
        t_part.T.reshape(N_SLICES, 2, P).transpose(1, 0, 2).reshape(2, N_SLICES * P))
    op2 = np.ascontiguousarray(
        o_part.T.reshape(N_SLICES, 2, P).transpose(1, 0, 2).reshape(2, N_SLICES * P))

    pidx = np.arange(P)
    gind = (pidx[None, :] // 2 == np.arange(GPC)[:, None]).astype(np.float32)

    rhs_t = np.zeros((KDIM, 512), dtype=np.float32)
    rhs_o = np.zeros((KDIM, 512), dtype=np.float32)
    rhs_t[:GPC, :256] = -t_groups
    rhs_t[:GPC, 256:] = -t_groups
    rhs_o[:GPC, :256] = -o_groups
    rhs_o[:GPC, 256:] = -o_groups
    rhs_t[GPC, :256] = 1.0
    rhs_t[GPC + 1, 256:] = 1.0
    rhs_o[GPC, :256] = 1.0
    rhs_o[GPC + 1, 256:] = 1.0

    return {
        "t_part2": tp2, "o_part2": op2, "g_ind": gind,
        "rhs_t": rhs_t, "rhs_o": rhs_o,
    }


def kernel(input, gdt_ts):
    input = np.asarray(input)
    gdt_ts = np.asarray(gdt_ts)
    o = input.reshape(-1)[: G_REAL * D].astype(np.float32, copy=False)
    t = gdt_ts.reshape(-1)[: G_REAL * D].astype(np.float32, copy=False)

    t_g = np.zeros((G_PAD, D), dtype=np.float32)
    o_g = np.zeros((G_PAD, D), dtype=np.float32)
    t_g[:G_REAL] = t.reshape(G_REAL, D)
    o_g[:G_REAL] = o.reshape(G_REAL, D)

    if "nc" not in _CACHED:
        _CACHED["nc"] = _build_program()
    nc = _CACHED["nc"]

    in_maps = []
    for c in range(N_CORES):
        sl = slice(c * GPC, (c + 1) * GPC)
        in_maps.append(_prep_core_inputs(t_g[sl], o_g[sl]))

    res = run_bass_kernel_spmd(nc, in_maps, list(range(N_CORES)))

    total = np.float64(0.0)
    for c in range(N_CORES):
        m_acc = res.results[c]["m_acc"].astype(np.float64)
        s_acc = res.results[c]["s_acc"].astype(np.float64)
        total += m_acc.sum() - CAP * s_acc.sum()

    loss = 2.0 * total / float(N_PAIRS)
    return np.array([loss], dtype=np.float32)


# revision 9
# speedup vs baseline: 1.9557x; 1.9557x over previous
"""Trainium2 Bass kernel for BatchRankingLoss.

Reference computation (B=131072, d=256 decoys, K=512 complexes, G=K-1=511 groups):
    o, t -> reshape to [G, d]
    dt = t_i - t_j ; y = sign-ish(dt); w = |dt| > 0.1
    dL = w * max(0, 1 + y*(o_i - o_j));  loss = sum(dL) / (G*d*(d-1))

Key identity used on device: dL is symmetric in (i,j) for |dt|>0.1 pairs, so
    sum(dL) = 2 * sum_{(i,j): dt_ij > 0.1} relu(1 + o_i - o_j)

Device computation per core (64 groups/core, group axis sharded over 8 cores):
  partition p in [0,128) = (g_local = p//2, half = p%2); per partition the free
  axis enumerates (i_local in [0,128), j in [0,256)) = 32768 elements, walked in
  32 chunks of [128, 1024] (each chunk = two N=512 matmul slices).

  - PE:  u   = (t_i - 0.1) - t_j  via K=66 block-diagonal matmul -> PSUM
         do1 = (1 + o_i) - o_j    via K=66 block-diagonal matmul -> PSUM
         (weights [66,128] per slice: rows 0-1 = t'/o' values for the slice's
          two i_local values, rows 2-65 = group-indicator; moving operand fixed
          [66,512]: rows 0-1 = i-slot indicators, rows 2-65 = -t_j / -o_j rows)
  - ACT: s = sigmoid(-1e9 * u)  (exact 0/1 mask away from a ~1e-8 band),
         accum_out -> sig_acc column
  - DVE: m = max(16*s, do1) via scalar_tensor_tensor, accum_out -> m_acc column
  Host:  sum(m) - 16*sum(s) == sum relu(do1 - 16*s) == masked hinge sum,
         since relu(x - S) = max(x, S) - S and s is exactly 0/1.

Host gathers the 8 cores' [128,32] accumulators: loss = 2*(sum m - 16*sum s)/N.
"""

import numpy as np
from contextlib import ExitStack

import concourse.bacc as bacc
import concourse.mybir as mybir
import concourse.tile as tile
from concourse.bass_utils import run_bass_kernel_spmd

N_CORES = 8
D = 256                 # decoys per complex
G_REAL = 511            # torch loop skips the final group
G_PAD = 512             # pad with a zero group so every core gets 64
GPC = G_PAD // N_CORES  # 64 groups per core
P = 128                 # partitions = GPC * 2 halves
IPB = 128               # i_local values per partition (= D/2)
N_SLICES = 64           # 512-wide matmul slices per sweep (2 i_local each)
N_CHUNKS = 32           # [128, 1024] chunks (2 slices each)
KDIM = GPC + 2          # matmul contraction: 64 group rows + 2 i-slot rows
BIG = 1.0e9
CAP = 16.0              # mask magnitude; > max |1 + o_i - o_j|
N_PAIRS = G_REAL * D * (D - 1)

_CACHED = {}


def _build_program(repeat=1):
    """Build the SPMD program. repeat>1 re-runs the compute loop in-NEFF
    (identical work, same outputs) for wall-clock delta timing."""
    nc = bacc.Bacc("TRN2", target_bir_lowering=False, debug=False,
                   num_devices=N_CORES)
    f32 = mybir.dt.float32

    # Per-core external inputs (host-prepped layouts).
    tp2 = nc.dram_tensor("t_part2", [2, N_SLICES * P], f32, kind="ExternalInput")
    op2 = nc.dram_tensor("o_part2", [2, N_SLICES * P], f32, kind="ExternalInput")
    gind = nc.dram_tensor("g_ind", [GPC, P], f32, kind="ExternalInput")
    rhs_t_d = nc.dram_tensor("rhs_t", [KDIM, 512], f32, kind="ExternalInput")
    rhs_o_d = nc.dram_tensor("rhs_o", [KDIM, 512], f32, kind="ExternalInput")

    m_acc_d = nc.dram_tensor("m_acc", [P, N_CHUNKS], f32, kind="ExternalOutput")
    s_acc_d = nc.dram_tensor("s_acc", [P, N_CHUNKS], f32, kind="ExternalOutput")

    with ExitStack() as ctx:
        tc = ctx.enter_context(tile.TileContext(nc, num_cores=N_CORES))
        consts = ctx.enter_context(tc.tile_pool(name="consts", bufs=1))
        psum_u = ctx.enter_context(tc.tile_pool(name="pu", bufs=2, space="PSUM"))
        psum_do = ctx.enter_context(tc.tile_pool(name="pdo", bufs=2, space="PSUM"))
        s_pool = ctx.enter_context(tc.tile_pool(name="sp", bufs=3))
        m_pool = ctx.enter_context(tc.tile_pool(name="mp", bufs=2))

        f32r = mybir.dt.float32r
        w_t = consts.tile([KDIM, N_SLICES * P], f32r)
        w_o = consts.tile([KDIM, N_SLICES * P], f32r)
        rhs_t = consts.tile([KDIM, 512], f32r)
        rhs_o = consts.tile([KDIM, 512], f32r)
        g_small = consts.tile([GPC, P], f32r)
        m_acc = consts.tile([P, N_CHUNKS], f32)
        s_acc = consts.tile([P, N_CHUNKS], f32)

        nc.sync.dma_start(g_small[:], gind[:].bitcast(f32r))
        nc.sync.dma_start(rhs_t[:], rhs_t_d[:].bitcast(f32r))
        nc.sync.dma_start(rhs_o[:], rhs_o_d[:].bitcast(f32r))
        nc.sync.dma_start(w_t[GPC:KDIM, :], tp2[:].bitcast(f32r))
        nc.sync.dma_start(w_o[GPC:KDIM, :], op2[:].bitcast(f32r))

        # Replicate the fixed group-indicator block across all 64 slice
        # positions of each weight buffer (stride-0 read AP), split in half so
        # early matmuls only wait on the first piece.
        HREP = N_SLICES // 2
        for hh in range(2):
            src = g_small[:, None, :].broadcast_to((GPC, HREP, P))
            dst_t = w_t[0:GPC, hh * HREP * P:(hh + 1) * HREP * P]
            dst_o = w_o[0:GPC, hh * HREP * P:(hh + 1) * HREP * P]
            nc.vector.tensor_copy(dst_t.rearrange("g (r p) -> g r p", p=P), src)
            nc.scalar.copy(dst_o.rearrange("g (r p) -> g r p", p=P), src)

        for c in range(N_CHUNKS * repeat):
            c = c % N_CHUNKS
            u_t = psum_u.tile([P, 1024], f32, tag="u")
            do_t = psum_do.tile([P, 1024], f32, tag="do")
            for h in range(2):
                s = 2 * c + h
                nc.tensor.matmul(
                    u_t[:, h * 512:(h + 1) * 512],
                    lhsT=w_t[:, s * P:(s + 1) * P],
                    rhs=rhs_t[:],
                    start=True, stop=True,
                )
            for h in range(2):
                s = 2 * c + h
                nc.tensor.matmul(
                    do_t[:, h * 512:(h + 1) * 512],
                    lhsT=w_o[:, s * P:(s + 1) * P],
                    rhs=rhs_o[:],
                    start=True, stop=True,
                )
            s_t = s_pool.tile([P, 1024], f32, tag="s")
            nc.scalar.activation(
                s_t[:], u_t[:], mybir.ActivationFunctionType.Sigmoid,
                scale=-BIG, accum_out=s_acc[:, c:c + 1],
            )
            m_t = m_pool.tile([P, 1024], f32, tag="m")
            nc.vector.scalar_tensor_tensor(
                out=m_t[:], in0=s_t[:], scalar=CAP, in1=do_t[:],
                op0=mybir.AluOpType.mult, op1=mybir.AluOpType.max,
                accum_out=m_acc[:, c:c + 1],
            )

        nc.sync.dma_start(m_acc_d[:], m_acc[:])
        nc.sync.dma_start(s_acc_d[:], s_acc[:])

    nc.compile()
    return nc


def _prep_core_inputs(t_groups, o_groups):
    """Build per-core input arrays from [GPC, D] group slabs (float32)."""
    tp = t_groups - np.float32(0.1)   # fold the -0.1 threshold into t_i
    op = o_groups + np.float32(1.0)   # fold the +1 hinge margin into o_i

    # [128, 128]: row p=(g_local*2+half) holds i_local values of that half.
    t_part = tp.reshape(GPC * 2, IPB)
    o_part = op.reshape(GPC * 2, IPB)
    # [2, N_SLICES*P]: row k, col s*P+p = value at (p, i_local=2s+k).
    tp2 = np.ascontiguousarray(
        t_part.T.reshape(N_SLICES, 2, P).transpose(1, 0, 2).reshape(2, N_SLICES * P))
    op2 = np.ascontiguousarray(
        o_part.T.reshape(N_SLICES, 2, P).transpose(1, 0, 2).reshape(2, N_SLICES * P))

    pidx = np.arange(P)
    gind = (pidx[None, :] // 2 == np.arange(GPC)[:, None]).astype(np.float32)

    rhs_t = np.zeros((KDIM, 512), dtype=np.float32)
    rhs_o = np.zeros((KDIM, 512), dtype=np.float32)
    rhs_t[:GPC, :256] = -t_groups
    rhs_t[:GPC, 256:] = -t_groups
    rhs_o[:GPC, :256] = -o_groups
    rhs_o[:GPC, 256:] = -o_groups
    rhs_t[GPC, :256] = 1.0
    rhs_t[GPC + 1, 256:] = 1.0
    rhs_o[GPC, :256] = 1.0
    rhs_o[GPC + 1, 256:] = 1.0

    return {
        "t_part2": tp2, "o_part2": op2, "g_ind": gind,
        "rhs_t": rhs_t, "rhs_o": rhs_o,
    }


def kernel(input, gdt_ts):
    input = np.asarray(input)
    gdt_ts = np.asarray(gdt_ts)
    o = input.reshape(-1)[: G_REAL * D].astype(np.float32, copy=False)
    t = gdt_ts.reshape(-1)[: G_REAL * D].astype(np.float32, copy=False)

    t_g = np.zeros((G_PAD, D), dtype=np.float32)
    o_g = np.zeros((G_PAD, D), dtype=np.float32)
    t_g[:G_REAL] = t.reshape(G_REAL, D)
    o_g[:G_REAL] = o.reshape(G_REAL, D)

    if "nc" not in _CACHED:
        _CACHED["nc"] = _build_program()
    nc = _CACHED["nc"]

    in_maps = []
    for c in range(N_CORES):
        sl = slice(c * GPC, (c + 1) * GPC)
        in_maps.append(_prep_core_inputs(t_g[sl], o_g[sl]))

    res = run_bass_kernel_spmd(nc, in_maps, list(range(N_CORES)))

    total = np.float64(0.0)
    for c in range(N_CORES):
        m_acc = res.results[c]["m_acc"].astype(np.float64)
        s_acc = res.results[c]["s_acc"].astype(np.float64)
        total += m_acc.sum() - CAP * s_acc.sum()

    loss = 2.0 * total / float(N_PAIRS)
    return np.array([loss], dtype=np.float32)


# revision 13
# speedup vs baseline: 6.5981x; 3.3737x over previous
"""Trainium2 Bass kernel for BatchRankingLoss.

Reference computation (B=131072, d=256 decoys, K=512 complexes, G=K-1=511 groups):
    o, t -> reshape to [G, d]
    dt = t_i - t_j ; y = sign-ish(dt) ; w = |dt| > 0.1
    dL = w * max(0, 1 + y*(o_i - o_j)) ; loss = sum(dL) / (G*d*(d-1))

Key identity used on device: dL is symmetric in (i,j) for |dt|>0.1 pairs, so
    sum(dL) = 2 * sum_{(i,j): dt_ij > 0.1} relu(1 + o_i - o_j)

Device computation per core (64 groups/core, group axis sharded over 8 cores):
  partition p in [0,128) = (g_local = p//2, half = p%2); per partition the free
  axis enumerates (i_local in [0,128), j in [0,256)) = 32768 elements, walked in
  32 chunks of [128, 1024] (each chunk = two N=512 matmul slices).

  - PE:  u   = (t_i - 0.1) - t_j  via K=66 block-diagonal float32r matmul -> PSUM
         do1 = (1 + o_i) - o_j    via K=66 block-diagonal float32r matmul -> PSUM
         (weights [66,128] per slice: rows 0-63 = group-indicator block, rows
          64-65 = t'/o' values for the slice's two i_local values; moving
          operand fixed [66,512]: rows 0-63 = -t_j/-o_j rows, 64-65 = i-slot
          indicators)
  - ACT: h = relu(do1)  (PSUM -> SBUF)
  - DVE: m = (u is_gt 0) * h  via scalar_tensor_tensor (exact mask),
         accum_out -> m_acc column
  Host:  loss = 2 * sum(m_acc over all cores) / N
"""

import numpy as np
from contextlib import ExitStack

import concourse.bacc as bacc
import concourse.mybir as mybir
import concourse.tile as tile
from concourse.bass_utils import run_bass_kernel_spmd

N_CORES = 8
D = 256                 # decoys per complex
G_REAL = 511            # torch loop skips the final group
G_PAD = 512             # pad with a zero group so every core gets 64
GPC = G_PAD // N_CORES  # 64 groups per core
P = 128                 # partitions = GPC * 2 halves
IPB = 128               # i_local values per partition (= D/2)
N_SLICES = 64           # 512-wide matmul slices per sweep (2 i_local each)
N_CHUNKS = 32           # [128, 1024] chunks (2 slices each)
KDIM = GPC + 2          # matmul contraction: 64 group rows + 2 i-slot rows
N_PAIRS = G_REAL * D * (D - 1)

_CACHED = {}


def _build_program(repeat=1, mode="full"):
    """Build the SPMD program. repeat>1 re-runs the compute loop in-NEFF
    (identical work, same outputs) for wall-clock delta timing. mode
    ("mm"|"mm_act"|"mm_dve"|"full") strips pipeline stages for perf
    diagnosis (outputs are garbage except in "full")."""
    nc = bacc.Bacc("TRN2", target_bir_lowering=False, debug=False,
                   num_devices=N_CORES)
    f32 = mybir.dt.float32

    # Per-core external inputs (host-prepped layouts).
    tp2 = nc.dram_tensor("t_part2", [2, N_SLICES * P], f32, kind="ExternalInput")
    op2 = nc.dram_tensor("o_part2", [2, N_SLICES * P], f32, kind="ExternalInput")
    gind = nc.dram_tensor("g_ind", [GPC, P], f32, kind="ExternalInput")
    rhs_t_d = nc.dram_tensor("rhs_t", [KDIM, 512], f32, kind="ExternalInput")
    rhs_o_d = nc.dram_tensor("rhs_o", [KDIM, 512], f32, kind="ExternalInput")

    m_acc_d = nc.dram_tensor("m_acc", [P, N_CHUNKS], f32, kind="ExternalOutput")

    with ExitStack() as ctx:
        tc = ctx.enter_context(tile.TileContext(nc, num_cores=N_CORES))
        consts = ctx.enter_context(tc.tile_pool(name="consts", bufs=1))
        psum_u = ctx.enter_context(tc.tile_pool(name="pu", bufs=2, space="PSUM"))
        psum_do = ctx.enter_context(tc.tile_pool(name="pdo", bufs=2, space="PSUM"))
        h_pool = ctx.enter_context(tc.tile_pool(name="hp", bufs=4))
        m_pool = ctx.enter_context(tc.tile_pool(name="mp", bufs=4))

        f32r = mybir.dt.float32r
        w_t = consts.tile([KDIM, N_SLICES * P], f32r)
        w_o = consts.tile([KDIM, N_SLICES * P], f32r)
        rhs_t = consts.tile([KDIM, 512], f32r)
        rhs_o = consts.tile([KDIM, 512], f32r)
        g_small = consts.tile([GPC, P], f32r)
        m_acc = consts.tile([P, N_CHUNKS], f32)

        nc.sync.dma_start(g_small[:], gind[:].bitcast(f32r))
        nc.sync.dma_start(rhs_t[:], rhs_t_d[:].bitcast(f32r))
        nc.sync.dma_start(rhs_o[:], rhs_o_d[:].bitcast(f32r))
        nc.sync.dma_start(w_t[GPC:KDIM, :], tp2[:].bitcast(f32r))
        nc.sync.dma_start(w_o[GPC:KDIM, :], op2[:].bitcast(f32r))

        # Replicate the fixed group-indicator block across all 64 slice
        # positions of each weight buffer (stride-0 read AP), split in half so
        # early matmuls only wait on the first piece.
        HREP = N_SLICES // 2
        for hh in range(2):
            src = g_small[:, None, :].broadcast_to((GPC, HREP, P))
            dst_t = w_t[0:GPC, hh * HREP * P:(hh + 1) * HREP * P]
            dst_o = w_o[0:GPC, hh * HREP * P:(hh + 1) * HREP * P]
            nc.vector.tensor_copy(dst_t.rearrange("g (r p) -> g r p", p=P), src)
            nc.scalar.copy(dst_o.rearrange("g (r p) -> g r p", p=P), src)

        for c in range(N_CHUNKS * repeat):
            c = c % N_CHUNKS
            u_t = psum_u.tile([P, 1024], f32, tag="u")
            do_t = psum_do.tile([P, 1024], f32, tag="do")
            for h in range(2):
                s = 2 * c + h
                nc.tensor.matmul(
                    u_t[:, h * 512:(h + 1) * 512],
                    lhsT=w_t[:, s * P:(s + 1) * P],
                    rhs=rhs_t[:],
                    start=True, stop=True,
                )
            for h in range(2):
                s = 2 * c + h
                nc.tensor.matmul(
                    do_t[:, h * 512:(h + 1) * 512],
                    lhsT=w_o[:, s * P:(s + 1) * P],
                    rhs=rhs_o[:],
                    start=True, stop=True,
                )
            if mode == "mm":
                continue
            if mode == "mm_dve":
                m_t = m_pool.tile([P, 1024], f32, tag="m")
                nc.vector.tensor_scalar(
                    out=m_t[:], in0=u_t[:], scalar1=0.0, scalar2=1.0,
                    op0=mybir.AluOpType.is_gt, op1=mybir.AluOpType.mult,
                    accum_out=m_acc[:, c:c + 1],
                )
                continue
            h_t = h_pool.tile([P, 1024], f32, tag="h")
            nc.scalar.activation(
                h_t[:], do_t[:], mybir.ActivationFunctionType.Relu,
            )
            if mode == "mm_act":
                continue
            m_t = m_pool.tile([P, 1024], f32, tag="m")
            nc.vector.scalar_tensor_tensor(
                out=m_t[:], in0=u_t[:], scalar=0.0, in1=h_t[:],
                op0=mybir.AluOpType.is_gt, op1=mybir.AluOpType.mult,
                accum_out=m_acc[:, c:c + 1],
            )

        if mode in ("full", "mm_dve"):
            nc.sync.dma_start(m_acc_d[:], m_acc[:])

    nc.compile()
    return nc


def _prep_core_inputs(t_groups, o_groups):
    """Build per-core input arrays from [GPC, D] group slabs (float32)."""
    tp = t_groups - np.float32(0.1)   # fold the -0.1 threshold into t_i
    op = o_groups + np.float32(1.0)   # fold the +1 hinge margin into o_i

    # [128, 128]: row p=(g_local*2+half) holds i_local values of that half.
    t_part = tp.reshape(GPC * 2, IPB)
    o_part = op.reshape(GPC * 2, IPB)
    # [2, N_SLICES*P]: row k, col s*P+p = value at (p, i_local=2s+k).
    tp2 = np.ascontiguousarray(
        t_part.T.reshape(N_SLICES, 2, P).transpose(1, 0, 2).reshape(2, N_SLICES * P))
    op2 = np.ascontiguousarray(
        o_part.T.reshape(N_SLICES, 2, P).transpose(1, 0, 2).reshape(2, N_SLICES * P))

    pidx = np.arange(P)
    gind = (pidx[None, :] // 2 == np.arange(GPC)[:, None]).astype(np.float32)

    rhs_t = np.zeros((KDIM, 512), dtype=np.float32)
    rhs_o = np.zeros((KDIM, 512), dtype=np.float32)
    rhs_t[:GPC, :256] = -t_groups
    rhs_t[:GPC, 256:] = -t_groups
    rhs_o[:GPC, :256] = -o_groups
    rhs_o[:GPC, 256:] = -o_groups
    rhs_t[GPC, :256] = 1.0
    rhs_t[GPC + 1, 256:] = 1.0
    rhs_o[GPC, :256] = 1.0
    rhs_o[GPC + 1, 256:] = 1.0

    return {
        "t_part2": tp2, "o_part2": op2, "g_ind": gind,
        "rhs_t": rhs_t, "rhs_o": rhs_o,
    }


def kernel(input, gdt_ts):
    input = np.asarray(input)
    gdt_ts = np.asarray(gdt_ts)
    o = input.reshape(-1)[: G_REAL * D].astype(np.float32, copy=False)
    t = gdt_ts.reshape(-1)[: G_REAL * D].astype(np.float32, copy=False)

    t_g = np.zeros((G_PAD, D), dtype=np.float32)
    o_g = np.zeros((G_PAD, D), dtype=np.float32)
    t_g[:G_REAL] = t.reshape(G_REAL, D)
    o_g[:G_REAL] = o.reshape(G_REAL, D)

    if "nc" not in _CACHED:
        _CACHED["nc"] = _build_program()
    nc = _CACHED["nc"]

    in_maps = []
    for c in range(N_CORES):
        sl = slice(c * GPC, (c + 1) * GPC)
        in_maps.append(_prep_core_inputs(t_g[sl], o_g[sl]))

    res = run_bass_kernel_spmd(nc, in_maps, list(range(N_CORES)))

    total = np.float64(0.0)
    for c in range(N_CORES):
        total += res.results[c]["m_acc"].astype(np.float64).sum()

    loss = 2.0 * total / float(N_PAIRS)
    return np.array([loss], dtype=np.float32)
